# revision 24
# baseline (speedup 1.0000x reference)
"""Trainium2 Bass kernel for dual (spectral + spatial) multi-head cross-attention.

Reference computation (see problem):
  q, kv: [B=2, S=32, H=24, W=24, C=64], heads=4, head_dim=16, scale=0.25
  spectral: attention over S per (b, pixel, head)
  spatial:  attention over H*W per (b, band, head)
  out = x_spectral + x_spatial

Sharding (8 cores):
  spatial : (b, s) pairs, 64 total -> 8 per core
  spectral: (b, pixel) pairs, 1152 total -> 144 per core (x4 heads = 576 problems)

Device strategy (per core), all matmul operands bf16:
  spatial per (b,s): t and p padded 576->640 (5 chunks of 128 / p-chunks 512+128).
    Q/K live at SBUF partitions 32h..32h+15 (head h) so the 4 heads' score
    matmuls run as concurrent 32x128 PE row-tiles into separate PSUM banks
    (h -> bank h for the 512-chunk, bank 4 for the 128-chunk).  exp via ACT
    (scale folded) -> bf16 pt.  PV uses V+ones lhsT (17 cols) with out at
    PSUM partitions 32h (bank 5) so the 4 heads run as concurrent 128x32
    col-tiles; the ones column yields softmax denominators for free.
    Host divides + transposes.
  spectral: 4 problems ([32,16] attention) packed block-diagonally into one
    [21,128]x[21,128]->[128,128] matmul with mask rows baked into the operands
    (exp(-30) ~ 0 kills cross-problem terms); PV = [128,128]x[128,18] with
    block-diag V+ones.  Fixed PSUM banks 6,7.  Host extracts diagonal blocks.
"""
import sys

sys.path.insert(0, '/opt/trn_rl_repo')

import numpy as np

B, S, HH, WW, C = 2, 32, 24, 24, 64
NH, HD = 4, 16
SCALE = HD ** -0.5
HW = HH * WW                      # 576
HWP = 640                         # padded t / p extent (5 x 128)
NCORES = 8
NS = (B * S) // NCORES            # 8 spatial (b,s) problems per core
NPX = (B * HW) // NCORES          # 144 (b,pixel) pairs per core
NQ = NPX * NH                     # 576 spectral problems per core
NG = NQ // 4                      # 144 block-diag groups of 4 problems
NHB = NG // 8                     # 18 spectral half-batches (8 groups each)
MASK = 120.0                      # pre-scale mask magnitude (exp(-30) after scale)

PCW = [256, 256, 128]             # spatial p-chunks (sum 640, >=576 real)

_CACHE = {}


def _bf16(a):
    import ml_dtypes
    return np.asarray(a, dtype=np.float32).astype(ml_dtypes.bfloat16)


# ---------------------------------------------------------------- host prep

def _prep_spatial(q2, kv2):
    """q2, kv2: [NS, HW, C] fp32 for this core's (b,s) slice.
    Returns kt [NS,128,2,HWP], qt [NS,128,2,HWP], vo [NS,5,128,NH*17] (bf16).
    kt[i, pb+d, blk, t] = K_h[t, d] (pb,blk per head);  qt same for Q;
    vo[i, tc, r, 17h+j] = V_h[128tc+r, j], ones at col 17h+16 (0 on pad rows).
    """
    ns = q2.shape[0]
    kt = np.zeros((ns, 128, 2, HWP), np.float32)
    qt = np.zeros((ns, 128, 2, HWP), np.float32)
    vo = np.zeros((ns, 5, 128, NH * 17), np.float32)
    kr = kv2.reshape(ns, HW, NH, HD)
    qr = q2.reshape(ns, HW, NH, HD)
    for h in range(NH):
        # PE row bands: heads 0-2 at partitions 32h (block 0), head 3
        # shares band 0 (block 1) — partition base 96 is unusable.
        pb, blk = (32 * h, 0) if h < 3 else (0, 1)
        kt[:, pb:pb + 16, blk, :HW] = kr[:, :, h, :].transpose(0, 2, 1)
        qt[:, pb:pb + 16, blk, :HW] = qr[:, :, h, :].transpose(0, 2, 1)
    for tc in range(5):
        t0, t1 = 128 * tc, min(128 * (tc + 1), HW)
        tw = t1 - t0
        blk = kr[:, t0:t1, :, :]                      # [ns,tw,h,d]
        for h in range(NH):
            vo[:, tc, :tw, 17 * h:17 * h + 16] = blk[:, :, h, :]
            vo[:, tc, :tw, 17 * h + 16] = 1.0
    return _bf16(kt), _bf16(qt), _bf16(vo)


def _prep_spectral(q1, kv1):
    """q1, kv1: [NPX, S, C] fp32 for this core's (b,px) slice.
    Problems q = px*4 + h; groups of 4 -> block-diag operands.
    Returns km [NG,21,128], qm [NG,21,128], vom [NG,128,18] (bf16)."""
    npx = q1.shape[0]
    kq = kv1.reshape(npx, S, NH, HD).transpose(0, 2, 1, 3).reshape(NQ, S, HD)
    qq = q1.reshape(npx, S, NH, HD).transpose(0, 2, 1, 3).reshape(NQ, S, HD)
    km = np.zeros((NG, 21, 128), np.float32)
    qm = np.zeros((NG, 21, 128), np.float32)
    vom = np.zeros((NG, 128, 18), np.float32)
    for j in range(4):
        # lhsT rows 0..15: d, cols j*32+t = K[t, d]
        km[:, :16, j * 32:(j + 1) * 32] = kq[j::4].transpose(0, 2, 1)
        qm[:, :16, j * 32:(j + 1) * 32] = qq[j::4].transpose(0, 2, 1)
        # mask rows: scores^T[m=t-stack, n=s-stack] += sum_i lhsT[16+i,m]*rhs[16+i,n]
        km[:, 16 + j, j * 32:(j + 1) * 32] = MASK      # lhsT indicator * M
        qm[:, 16 + j, j * 32:(j + 1) * 32] = 1.0       # rhs indicator
        km[:, 20, :] = -MASK                           # constant -M row
        qm[:, 20, :] = 1.0
        vom[:, j * 32:(j + 1) * 32, :16] = kq[j::4]
        vom[:, j * 32:(j + 1) * 32, 16] = 1.0
    return _bf16(km), _bf16(qm), _bf16(vom)


def _host_slices(query, feat):
    """Full inputs -> per-core input dicts (numpy)."""
    q4 = query.reshape(B * S, HW, C)        # (b,s) major
    f4 = feat.reshape(B * S, HW, C)
    q1 = query.transpose(0, 2, 3, 1, 4).reshape(B * HW, S, C)  # (b,px) major
    f1 = feat.transpose(0, 2, 3, 1, 4).reshape(B * HW, S, C)
    maps = []
    for c in range(NCORES):
        kt, qt, vo = _prep_spatial(q4[c * NS:(c + 1) * NS],
                                   f4[c * NS:(c + 1) * NS])
        km, qm, vom = _prep_spectral(q1[c * NPX:(c + 1) * NPX],
                                     f1[c * NPX:(c + 1) * NPX])
        maps.append(dict(kt=kt, qt=qt, vo=vo, km=km, qm=qm, vom=vom))
    return maps


def _decode(results):
    """Per-core outputs -> full [B,S,H,W,C]."""
    x_spat = np.zeros((B * S, HW, C), np.float32)
    x_spec = np.zeros((B * HW, S, C), np.float32)
    for c, r in enumerate(results):
        ospat = r["out_spat"]          # [NS, 128, HW]
        for h in range(NH):
            num = ospat[:, 32 * h:32 * h + 16, :]          # [NS, 16, 576]
            den = ospat[:, 32 * h + 16:32 * h + 17, :]     # [NS, 1, 576]
            x_spat[c * NS:(c + 1) * NS, :, 16 * h:16 * h + 16] = \
                (num / den).transpose(0, 2, 1)
        ospec = r["out_spec"]          # [NG, 128, 17]
        for j in range(4):
            # group g block j = (pixel g, head j)
            blk = ospec[:, j * 32:(j + 1) * 32, :]
            num = blk[:, :, :16]                            # [NG, 32, 16]
            den = blk[:, :, 16:17]
            x_spec[c * NPX:(c + 1) * NPX, :, 16 * j:16 * j + 16] = num / den
    full_spat = x_spat.reshape(B, S, HH, WW, C)
    full_spec = x_spec.reshape(B, HH, WW, S, C).transpose(0, 3, 1, 2, 4)
    return full_spat + full_spec


# ------------------------------------------------------------- device build

def _build_program():
    import concourse.bacc as bacc
    import concourse.tile as tile
    from concourse import mybir

    FP32 = mybir.dt.float32
    BF16 = mybir.dt.bfloat16
    EXP = mybir.ActivationFunctionType.Exp

    from concourse.bass import _add_dep_helper

    nc = bacc.Bacc("TRN2", target_bir_lowering=False, debug=False)

    d_kt = nc.dram_tensor("kt", [NS, 128, 2, HWP], BF16, kind="ExternalInput")
    d_qt = nc.dram_tensor("qt", [NS, 128, 2, HWP], BF16, kind="ExternalInput")
    d_vo = nc.dram_tensor("vo", [NS, 5, 128, NH * 17], BF16, kind="ExternalInput")
    d_km = nc.dram_tensor("km", [NG, 21, 128], BF16, kind="ExternalInput")
    d_qm = nc.dram_tensor("qm", [NG, 21, 128], BF16, kind="ExternalInput")
    d_vom = nc.dram_tensor("vom", [NG, 128, 18], BF16, kind="ExternalInput")
    d_ospat = nc.dram_tensor("out_spat", [NS, 128, HW], FP32, kind="ExternalOutput")
    d_ospec = nc.dram_tensor("out_spec", [NG, 128, 17], FP32, kind="ExternalOutput")

    with tile.TileContext(nc) as tc:
        with (
            tc.tile_pool(name="cons", bufs=1) as cons,
            tc.tile_pool(name="io", bufs=3) as io,
            tc.tile_pool(name="spin", bufs=4) as spin,
            tc.tile_pool(name="pt", bufs=3) as ptp,
            tc.tile_pool(name="ps", bufs=1, space="PSUM") as ps,
        ):
            # persistent spatial V/ones operand
            vo_t = cons.tile([128, NS, 5, NH * 17], BF16)
            nc.sync.dma_start(vo_t[:], d_vo[:].rearrange("n tc p c -> p n tc c"))

            # all of PSUM as one tile.
            # banks 0-3: spatial scores (head h -> bank h, parity ping-pong
            #            in col halves 256*(n%2))
            # bank 4:    spare
            # bank 5:    spatial PV accumulators ([17]@partition 32h;
            #            pc0 -> cols 0-255, pc1 -> 256-511, pc2 -> 0-127)
            # banks 6,7: spectral scores / PV / out
            T = ps.tile([128, 8, 512], FP32, tag="T")

            kts, qts, sbos = {}, {}, {}
            _guard_init = {}
            kms, qms, voms = {}, {}, {}
            # explicit WAR guards: psum-region re-writers wait on the last
            # reader (ACT exp or DVE evacuation) of that region
            guards = {}

            def guarded(inst, key):
                dep = guards.pop(key, None)
                if dep is not None:
                    _add_dep_helper(inst.ins, dep.ins, sync=True,
                                    reason=f"WAR {key}")
                return inst

            def load_bs(i):
                kts[i] = io.tile([128, 2, HWP], BF16, tag="kt", name=f"kt{i}")
                qts[i] = io.tile([128, 2, HWP], BF16, tag="qt", name=f"qt{i}")
                nc.sync.dma_start(kts[i][:], d_kt[i])
                nc.sync.dma_start(qts[i][:], d_qt[i])
                sbos[i] = io.tile([128, HW], FP32, tag="so", name=f"so{i}")

            def load_hb(n):
                if n >= NHB:
                    return
                kms[n] = spin.tile([128, 8, 128], BF16, tag="km", name=f"km{n}")
                qms[n] = spin.tile([128, 8, 128], BF16, tag="qm", name=f"qm{n}")
                voms[n] = spin.tile([128, 8, 18], BF16, tag="vom", name=f"vom{n}")
                sl = np.s_[8 * n:8 * (n + 1)]
                nc.sync.dma_start(kms[n][0:21], d_km[sl].rearrange("g r c -> r g c"))
                nc.sync.dma_start(qms[n][0:21], d_qm[sl].rearrange("g r c -> r g c"))
                nc.sync.dma_start(voms[n][:], d_vom[sl].rearrange("g r c -> r g c"))

            # ---- spatial: chunk = (i, pc, tc); parity = n%2 ----
            # PV accumulator cols in bank 5 per pc:
            PVC = [0, 256, 0]                 # col base
            PVW = [256, 256, 128]             # accum width (pc2 incl junk)
            SBC = [0, 256, 512]               # sbos col base
            SBW = [256, 256, 64]              # real cols evacuated

            def spat_front(n, item):
                _, i, pc, t_c = item
                if pc == 1 and t_c == 0 and i + 1 < NS:
                    load_bs(i + 1)
                p0 = sum(PCW[:pc])
                pw = PCW[pc]
                par = n % 2
                for h in range(NH):
                    pb, blk = (32 * h, 0) if h < 3 else (0, 1)
                    mm = nc.tensor.matmul(
                        T[0:128, h, 256 * par:256 * par + pw],
                        kts[i][pb:pb + 16, blk, 128 * t_c:128 * (t_c + 1)],
                        qts[i][pb:pb + 16, blk, p0:p0 + pw],
                        start=True, stop=True, skip_group_check=True,
                        tile_position=(pb, 0))
                    if h == 0:
                        guarded(mm, f"s{par}")

            def spat_back(n, item):
                _, i, pc, t_c = item
                pw = PCW[pc]
                par = n % 2
                pt_t = ptp.tile([128, 4, 256], BF16, tag="pt", name=f"pt{n}")
                act = nc.scalar.activation(
                    pt_t[:, :, 0:pw],
                    T[0:128, 0:4, 256 * par:256 * par + pw],
                    EXP, scale=float(SCALE))
                guards[f"s{par}"] = act
                first = None
                for h in range(NH):
                    mm = nc.tensor.matmul(
                        T[32 * h:32 * h + 17, 5, PVC[pc]:PVC[pc] + PVW[pc]],
                        vo_t[0:128, i, t_c, 17 * h:17 * h + 17],
                        pt_t[0:128, h, 0:pw],
                        start=(t_c == 0), stop=(t_c == 4),
                        skip_group_check=True,
                        tile_position=(0, 32 * h))
                    if t_c == 0:
                        # start=True clears the whole bank's has_written bits;
                        # only h0 may clear, h1-3 ride on cleared bits but
                        # must issue after h0 (bank clear order).
                        if h == 0:
                            first = mm
                            guarded(mm, "pv5")
                        else:
                            _add_dep_helper(mm.ins, first.ins, sync=False,
                                            reason="bank clear order")
                if t_c == 4:
                    ev = nc.vector.tensor_copy(
                        sbos[i][0:128, SBC[pc]:SBC[pc] + SBW[pc]],
                        T[0:128, 5, PVC[pc]:PVC[pc] + SBW[pc]])
                    guards["pv5"] = ev
                    if pc == 2:
                        nc.sync.dma_start(d_ospat[i], sbos[i][:])

            # ---- spectral: half-batch hb of 8 groups in banks 6,7 ----

            def spec_front(n, item):
                hb = item[1]
                load_hb(hb + 3)
                for g in range(8):
                    mm = nc.tensor.matmul(
                        T[:, 6 + g // 4, (g % 4) * 128:(g % 4) * 128 + 128],
                        kms[hb][0:21, g, :], qms[hb][0:21, g, :],
                        start=True, stop=True, skip_group_check=True)
                    if g == 0:
                        guarded(mm, "spec6")
                    elif g == 4:
                        guarded(mm, "spec7")

            def spec_back(n, item):
                hb = item[1]
                pts = ptp.tile([128, 2, 512], BF16, tag="pts", name=f"pts{n}")
                act = nc.scalar.activation(pts[:], T[:, 6:8, :],
                                           EXP, scale=float(SCALE))
                guards["spec7"] = act
                for g in range(8):
                    # 18-col pitch keeps psum dst 8B-aligned
                    nc.tensor.matmul(
                        T[:, 6, 18 * g:18 * g + 18],
                        pts[:, g // 4, (g % 4) * 128:(g % 4) * 128 + 128],
                        voms[hb][:, g, :],
                        start=True, stop=True, skip_group_check=True)
                so_t = spin.tile([128, 8, 17], FP32, tag="spo", name=f"spo{n}")
                guards["spec6"] = nc.vector.tensor_copy(
                    so_t[:], T[:, 6, 0:144].rearrange(
                        "p (g c) -> p g c", c=18)[:, :, 0:17])
                nc.sync.dma_start(
                    d_ospec[8 * hb:8 * (hb + 1)].rearrange("g r c -> r g c"),
                    so_t[:])

            # ---- build interleaved chunk stream ----
            spat = [("spat", i, pc, t_c) for i in range(NS)
                    for pc in range(3) for t_c in range(5)]
            stream = []
            hb = 0
            for idx, itm in enumerate(spat):
                stream.append(itm)
                if hb < NHB and (idx + 1) * NHB >= (hb + 1) * len(spat):
                    stream.append(("spec", hb))
                    hb += 1
            while hb < NHB:
                stream.append(("spec", hb))
                hb += 1

            load_bs(0)
            for n in range(3):
                load_hb(n)
            # zero-fill PV bank once so evacuation of the unwritten
            # partition gaps (32h+17..32h+31) reads defined data
            guards["pv5"] = nc.vector.memset(T[0:128, 5, :], 0.0)

            fronts = {"spat": spat_front, "spec": spec_front}
            backs = {"spat": spat_back, "spec": spec_back}
            fronts[stream[0][0]](0, stream[0])
            for n in range(1, len(stream)):
                fronts[stream[n][0]](n, stream[n])
                backs[stream[n - 1][0]](n - 1, stream[n - 1])
            n = len(stream) - 1
            backs[stream[n][0]](n, stream[n])

    nc.compile()
    return nc


# ------------------------------------------------------------------ entry

def kernel(query, feat):
    from concourse.bass_utils import run_bass_kernel_spmd

    query = np.asarray(query, dtype=np.float32)
    feat = np.asarray(feat, dtype=np.float32)
    if "nc" not in _CACHE:
        _CACHE["nc"] = _build_program()
    nc = _CACHE["nc"]
    in_maps = _host_slices(query, feat)
    res = run_bass_kernel_spmd(nc, in_maps, core_ids=list(range(NCORES)))
    _CACHE["exec_time_ns"] = res.exec_time_ns
    _CACHE["res"] = res
    return _decode(res.results)


# ---------------------------------------------------- numpy device mirror

def _simulate_core(m):
    """Mirror of the device program in numpy, from prepped inputs to outputs."""
    kt = m["kt"].astype(np.float32)
    qt = m["qt"].astype(np.float32)
    vo = m["vo"].astype(np.float32)
    out_spat = np.zeros((NS, 128, HW), np.float32)
    for i in range(NS):
        for h in range(NH):
            pb, blk = (32 * h, 0) if h < 3 else (0, 1)
            sT = kt[i, pb:pb + 16, blk, :].T @ qt[i, pb:pb + 16, blk, :HW]
            pT = np.exp(SCALE * sT).astype(np.float32)   # [tpad, p]
            import ml_dtypes
            pT = pT.astype(ml_dtypes.bfloat16).astype(np.float32)
            acc = np.zeros((17, HW), np.float32)
            for t_c in range(5):
                acc += (vo[i, t_c, :, 17 * h:17 * h + 17].T
                        @ pT[128 * t_c:128 * (t_c + 1)])
            out_spat[i, 32 * h:32 * h + 17] = acc
    out_spec = np.zeros((NG, 128, 17), np.float32)  # 17 useful cols
    for g in range(NG):
        sT = m["km"][g].astype(np.float32).T @ m["qm"][g].astype(np.float32)
        pT = np.exp(SCALE * sT)
        import ml_dtypes
        pT = pT.astype(ml_dtypes.bfloat16).astype(np.float32)
        out_spec[g] = pT.T @ m["vom"][g].astype(np.float32)[:, :17]
    return dict(out_spat=out_spat, out_spec=out_spec)


def kernel_numpy(query, feat):
    """Host-only functional mirror (for layout validation)."""
    query = np.asarray(query, dtype=np.float32)
    feat = np.asarray(feat, dtype=np.float32)
    return _decode([_simulate_core(m) for m in _host_slices(query, feat)])


# revision 29
# speedup vs baseline: 1.1168x; 1.1168x over previous
"""Trainium2 Bass kernel for dual (spectral + spatial) multi-head cross-attention.

Reference computation (see problem):
  q, kv: [B=2, S=32, H=24, W=24, C=64], heads=4, head_dim=16, scale=0.25
  spectral: attention over S per (b, pixel, head)
  spatial:  attention over H*W per (b, band, head)
  out = x_spectral + x_spatial

Sharding (8 cores):
  spatial : (b, s) pairs, 64 total -> 8 per core
  spectral: (b, pixel) pairs, 1152 total -> 144 per core (x4 heads = 576 problems)

Device strategy (per core), all matmul operands bf16:
  spatial per (b,s): t and p padded 576->640 (5 chunks of 128 / p-chunks 512+128).
    Q/K live at SBUF partitions 32h..32h+15 (head h) so the 4 heads' score
    matmuls run as concurrent 32x128 PE row-tiles into separate PSUM banks
    (h -> bank h for the 512-chunk, bank 4 for the 128-chunk).  exp via ACT
    (scale folded) -> bf16 pt.  PV uses V+ones lhsT (17 cols) with out at
    PSUM partitions 32h (bank 5) so the 4 heads run as concurrent 128x32
    col-tiles; the ones column yields softmax denominators for free.
    Host divides + transposes.
  spectral: 4 problems ([32,16] attention) packed block-diagonally into one
    [21,128]x[21,128]->[128,128] matmul with mask rows baked into the operands
    (exp(-30) ~ 0 kills cross-problem terms); PV = [128,128]x[128,18] with
    block-diag V+ones.  Fixed PSUM banks 6,7.  Host extracts diagonal blocks.
"""
import sys

sys.path.insert(0, '/opt/trn_rl_repo')

import numpy as np

B, S, HH, WW, C = 2, 32, 24, 24, 64
NH, HD = 4, 16
SCALE = HD ** -0.5
HW = HH * WW                      # 576
HWP = 640                         # padded t / p extent (5 x 128)
NCORES = 8
NS = (B * S) // NCORES            # 8 spatial (b,s) problems per core
NPX = (B * HW) // NCORES          # 144 (b,pixel) pairs per core
NQ = NPX * NH                     # 576 spectral problems per core
NG = NQ // 4                      # 144 block-diag groups of 4 problems
NHB = NG // 8                     # 18 spectral half-batches (8 groups each)
MASK = 120.0                      # pre-scale mask magnitude (exp(-30) after scale)

PCW = [256, 256, 128]             # spatial p-chunks (sum 640, >=576 real)

_CACHE = {}


def _bf16(a):
    import ml_dtypes
    return np.asarray(a, dtype=np.float32).astype(ml_dtypes.bfloat16)


# ---------------------------------------------------------------- host prep

def _prep_spatial(q2, kv2):
    """q2, kv2: [NS, HW, C] fp32 for this core's (b,s) slice.
    Returns kt [NS,128,2,HWP], qt [NS,128,2,HWP], vo [NS,5,128,NH*17] (bf16).
    kt[i, pb+d, blk, t] = K_h[t, d] (pb,blk per head);  qt same for Q;
    vo[i, tc, r, 17h+j] = V_h[128tc+r, j], ones at col 17h+16 (0 on pad rows).
    """
    ns = q2.shape[0]
    kt = np.zeros((ns, 128, 2, HWP), np.float32)
    qt = np.zeros((ns, 128, 2, HWP), np.float32)
    vo = np.zeros((ns, 5, 128, NH * 17), np.float32)
    kr = kv2.reshape(ns, HW, NH, HD)
    qr = q2.reshape(ns, HW, NH, HD)
    for h in range(NH):
        # PE row bands: heads 0-2 at partitions 32h (block 0), head 3
        # shares band 0 (block 1) — partition base 96 is unusable.
        pb, blk = (32 * h, 0) if h < 3 else (0, 1)
        kt[:, pb:pb + 16, blk, :HW] = kr[:, :, h, :].transpose(0, 2, 1)
        qt[:, pb:pb + 16, blk, :HW] = qr[:, :, h, :].transpose(0, 2, 1)
    for tc in range(5):
        t0, t1 = 128 * tc, min(128 * (tc + 1), HW)
        tw = t1 - t0
        blk = kr[:, t0:t1, :, :]                      # [ns,tw,h,d]
        for h in range(NH):
            vo[:, tc, :tw, 17 * h:17 * h + 16] = blk[:, :, h, :]
            vo[:, tc, :tw, 17 * h + 16] = 1.0
    return _bf16(kt), _bf16(qt), _bf16(vo)


def _prep_spectral(q1, kv1):
    """q1, kv1: [NPX, S, C] fp32 for this core's (b,px) slice.
    Problems q = px*4 + h; groups of 4 -> block-diag operands.
    Returns km [NG,21,128], qm [NG,21,128], vom [NG,128,18] (bf16)."""
    npx = q1.shape[0]
    kq = kv1.reshape(npx, S, NH, HD).transpose(0, 2, 1, 3).reshape(NQ, S, HD)
    qq = q1.reshape(npx, S, NH, HD).transpose(0, 2, 1, 3).reshape(NQ, S, HD)
    km = np.zeros((NG, 21, 128), np.float32)
    qm = np.zeros((NG, 21, 128), np.float32)
    vom = np.zeros((NG, 128, 18), np.float32)
    for j in range(4):
        # lhsT rows 0..15: d, cols j*32+t = K[t, d]
        km[:, :16, j * 32:(j + 1) * 32] = kq[j::4].transpose(0, 2, 1)
        qm[:, :16, j * 32:(j + 1) * 32] = qq[j::4].transpose(0, 2, 1)
        # mask rows: scores^T[m=t-stack, n=s-stack] += sum_i lhsT[16+i,m]*rhs[16+i,n]
        km[:, 16 + j, j * 32:(j + 1) * 32] = MASK      # lhsT indicator * M
        qm[:, 16 + j, j * 32:(j + 1) * 32] = 1.0       # rhs indicator
        km[:, 20, :] = -MASK                           # constant -M row
        qm[:, 20, :] = 1.0
        vom[:, j * 32:(j + 1) * 32, :16] = kq[j::4]
        vom[:, j * 32:(j + 1) * 32, 16] = 1.0
    return _bf16(km), _bf16(qm), _bf16(vom)


def _host_slices(query, feat):
    """Full inputs -> per-core input dicts (numpy)."""
    q4 = query.reshape(B * S, HW, C)        # (b,s) major
    f4 = feat.reshape(B * S, HW, C)
    q1 = query.transpose(0, 2, 3, 1, 4).reshape(B * HW, S, C)  # (b,px) major
    f1 = feat.transpose(0, 2, 3, 1, 4).reshape(B * HW, S, C)
    maps = []
    for c in range(NCORES):
        kt, qt, vo = _prep_spatial(q4[c * NS:(c + 1) * NS],
                                   f4[c * NS:(c + 1) * NS])
        km, qm, vom = _prep_spectral(q1[c * NPX:(c + 1) * NPX],
                                     f1[c * NPX:(c + 1) * NPX])
        maps.append(dict(kt=kt, qt=qt, vo=vo, km=km, qm=qm, vom=vom))
    return maps


def _decode(results):
    """Per-core outputs -> full [B,S,H,W,C]."""
    x_spat = np.zeros((B * S, HW, C), np.float32)
    x_spec = np.zeros((B * HW, S, C), np.float32)
    for c, r in enumerate(results):
        ospat = r["out_spat"]          # [NS, 128, HW]
        for h in range(NH):
            num = ospat[:, 32 * h:32 * h + 16, :]          # [NS, 16, 576]
            den = ospat[:, 32 * h + 16:32 * h + 17, :]     # [NS, 1, 576]
            x_spat[c * NS:(c + 1) * NS, :, 16 * h:16 * h + 16] = \
                (num / den).transpose(0, 2, 1)
        ospec = r["out_spec"]          # [NG, 128, 17]
        for j in range(4):
            # group g block j = (pixel g, head j)
            blk = ospec[:, j * 32:(j + 1) * 32, :]
            num = blk[:, :, :16]                            # [NG, 32, 16]
            den = blk[:, :, 16:17]
            x_spec[c * NPX:(c + 1) * NPX, :, 16 * j:16 * j + 16] = num / den
    full_spat = x_spat.reshape(B, S, HH, WW, C)
    full_spec = x_spec.reshape(B, HH, WW, S, C).transpose(0, 3, 1, 2, 4)
    return full_spat + full_spec


# ------------------------------------------------------------- device build

def _build_program():
    import concourse.bacc as bacc
    import concourse.tile as tile
    from concourse import mybir

    FP32 = mybir.dt.float32
    BF16 = mybir.dt.bfloat16
    EXP = mybir.ActivationFunctionType.Exp

    from concourse.bass import _add_dep_helper

    nc = bacc.Bacc("TRN2", target_bir_lowering=False, debug=False)

    d_kt = nc.dram_tensor("kt", [NS, 128, 2, HWP], BF16, kind="ExternalInput")
    d_qt = nc.dram_tensor("qt", [NS, 128, 2, HWP], BF16, kind="ExternalInput")
    d_vo = nc.dram_tensor("vo", [NS, 5, 128, NH * 17], BF16, kind="ExternalInput")
    d_km = nc.dram_tensor("km", [NG, 21, 128], BF16, kind="ExternalInput")
    d_qm = nc.dram_tensor("qm", [NG, 21, 128], BF16, kind="ExternalInput")
    d_vom = nc.dram_tensor("vom", [NG, 128, 18], BF16, kind="ExternalInput")
    d_ospat = nc.dram_tensor("out_spat", [NS, 128, HW], FP32, kind="ExternalOutput")
    d_ospec = nc.dram_tensor("out_spec", [NG, 128, 17], FP32, kind="ExternalOutput")

    with tile.TileContext(nc) as tc:
        with (
            tc.tile_pool(name="cons", bufs=1) as cons,
            tc.tile_pool(name="io", bufs=3) as io,
            tc.tile_pool(name="spin", bufs=4) as spin,
            tc.tile_pool(name="pt", bufs=6) as ptp,
            tc.tile_pool(name="ps", bufs=1, space="PSUM") as ps,
        ):
            # persistent spatial V/ones operand
            vo_t = cons.tile([128, NS, 5, NH * 17], BF16)
            nc.sync.dma_start(vo_t[:], d_vo[:].rearrange("n tc p c -> p n tc c"))

            # all of PSUM as one tile.
            # banks 0-3: spatial scores (head h -> bank h, parity ping-pong
            #            in col halves 256*(n%2))
            # bank 4:    spare
            # bank 5:    spatial PV accumulators ([17]@partition 32h;
            #            pc0 -> cols 0-255, pc1 -> 256-511, pc2 -> 0-127)
            # banks 6,7: spectral scores / PV / out
            T = ps.tile([128, 8, 512], FP32, tag="T")

            kts, qts, sbos = {}, {}, {}
            _guard_init = {}
            kms, qms, voms = {}, {}, {}
            # explicit WAR guards: psum-region re-writers wait on the last
            # reader (ACT exp or DVE evacuation) of that region
            guards = {}

            def guarded(inst, key):
                dep = guards.pop(key, None)
                if dep is not None:
                    _add_dep_helper(inst.ins, dep.ins, sync=True,
                                    reason=f"WAR {key}")
                return inst

            def load_bs(i):
                kts[i] = io.tile([128, 2, HWP], BF16, tag="kt", name=f"kt{i}")
                qts[i] = io.tile([128, 2, HWP], BF16, tag="qt", name=f"qt{i}")
                nc.sync.dma_start(kts[i][:], d_kt[i])
                nc.sync.dma_start(qts[i][:], d_qt[i])
                sbos[i] = io.tile([128, HW], FP32, tag="so", name=f"so{i}")

            def load_hb(n):
                if n >= NHB:
                    return
                kms[n] = spin.tile([128, 8, 128], BF16, tag="km", name=f"km{n}")
                qms[n] = spin.tile([128, 8, 128], BF16, tag="qm", name=f"qm{n}")
                voms[n] = spin.tile([128, 8, 18], BF16, tag="vom", name=f"vom{n}")
                sl = np.s_[8 * n:8 * (n + 1)]
                nc.sync.dma_start(kms[n][0:21], d_km[sl].rearrange("g r c -> r g c"))
                nc.sync.dma_start(qms[n][0:21], d_qm[sl].rearrange("g r c -> r g c"))
                nc.sync.dma_start(voms[n][:], d_vom[sl].rearrange("g r c -> r g c"))

            # ---- spatial: chunk = (i, pc, tc); parity = n%2 ----
            # PV accumulator cols in bank 5 per pc:
            PVC = [0, 256, 0]                 # col base
            PVW = [256, 256, 128]             # accum width (pc2 incl junk)
            SBC = [0, 256, 512]               # sbos col base
            SBW = [256, 256, 64]              # real cols evacuated

            def spat_front(n, item):
                _, i, pc, t_c = item
                if pc == 1 and t_c == 0 and i + 1 < NS:
                    load_bs(i + 1)
                p0 = sum(PCW[:pc])
                pw = PCW[pc]
                par = n % 2
                for h in range(NH):
                    pb, blk = (32 * h, 0) if h < 3 else (0, 1)
                    mm = nc.tensor.matmul(
                        T[0:128, h, 256 * par:256 * par + pw],
                        kts[i][pb:pb + 16, blk, 128 * t_c:128 * (t_c + 1)],
                        qts[i][pb:pb + 16, blk, p0:p0 + pw],
                        start=True, stop=True, skip_group_check=True,
                        tile_position=(pb, 0))
                    if h == 0:
                        guarded(mm, f"s{par}")

            pts_of = {}

            def spat_mid(n, item):
                _, i, pc, t_c = item
                pw = PCW[pc]
                par = n % 2
                pt_t = ptp.tile([128, 4, 256], BF16, tag="pt", name=f"pt{n}")
                pts_of[n] = pt_t
                act = nc.scalar.activation(
                    pt_t[:, :, 0:pw],
                    T[0:128, 0:4, 256 * par:256 * par + pw],
                    EXP, scale=float(SCALE))
                guards[f"s{par}"] = act

            def spat_tail(n, item):
                _, i, pc, t_c = item
                pw = PCW[pc]
                pt_t = pts_of.pop(n)
                first = None
                for h in range(NH):
                    mm = nc.tensor.matmul(
                        T[32 * h:32 * h + 17, 5, PVC[pc]:PVC[pc] + PVW[pc]],
                        vo_t[0:128, i, t_c, 17 * h:17 * h + 17],
                        pt_t[0:128, h, 0:pw],
                        start=(t_c == 0), stop=(t_c == 4),
                        skip_group_check=True,
                        tile_position=(0, 32 * h))
                    if t_c == 0:
                        # start=True clears the whole bank's has_written bits;
                        # only h0 may clear, h1-3 ride on cleared bits but
                        # must issue after h0 (bank clear order).
                        if h == 0:
                            first = mm
                            guarded(mm, "pv5")
                        else:
                            _add_dep_helper(mm.ins, first.ins, sync=False,
                                            reason="bank clear order")
                if t_c == 4:
                    ev = nc.vector.tensor_copy(
                        sbos[i][0:128, SBC[pc]:SBC[pc] + SBW[pc]],
                        T[0:128, 5, PVC[pc]:PVC[pc] + SBW[pc]])
                    guards["pv5"] = ev
                    if pc == 2:
                        nc.sync.dma_start(d_ospat[i], sbos[i][:])

            # ---- spectral: half-batch hb of 8 groups in banks 6,7 ----

            def spec_front(n, item):
                hb = item[1]
                load_hb(hb + 3)
                for g in range(8):
                    mm = nc.tensor.matmul(
                        T[:, 6 + g // 4, (g % 4) * 128:(g % 4) * 128 + 128],
                        kms[hb][0:21, g, :], qms[hb][0:21, g, :],
                        start=True, stop=True, skip_group_check=True)
                    if g == 0:
                        guarded(mm, "spec6")
                    elif g == 4:
                        guarded(mm, "spec7")

            def spec_mid(n, item):
                pts = ptp.tile([128, 2, 512], BF16, tag="pts", name=f"pts{n}")
                pts_of[n] = pts
                act = nc.scalar.activation(pts[:], T[:, 6:8, :],
                                           EXP, scale=float(SCALE))
                guards["spec7"] = act

            def spec_tail(n, item):
                hb = item[1]
                pts = pts_of.pop(n)
                for g in range(8):
                    # 18-col pitch keeps psum dst 8B-aligned
                    nc.tensor.matmul(
                        T[:, 6, 18 * g:18 * g + 18],
                        pts[:, g // 4, (g % 4) * 128:(g % 4) * 128 + 128],
                        voms[hb][:, g, :],
                        start=True, stop=True, skip_group_check=True)
                so_t = spin.tile([128, 8, 17], FP32, tag="spo", name=f"spo{n}")
                guards["spec6"] = nc.vector.tensor_copy(
                    so_t[:], T[:, 6, 0:144].rearrange(
                        "p (g c) -> p g c", c=18)[:, :, 0:17])
                nc.sync.dma_start(
                    d_ospec[8 * hb:8 * (hb + 1)].rearrange("g r c -> r g c"),
                    so_t[:])

            # ---- build interleaved chunk stream ----
            spat = [("spat", i, pc, t_c) for i in range(NS)
                    for pc in range(3) for t_c in range(5)]
            stream = []
            hb = 0
            for idx, itm in enumerate(spat):
                stream.append(itm)
                if hb < NHB and (idx + 1) * NHB >= (hb + 1) * len(spat):
                    stream.append(("spec", hb))
                    hb += 1
            while hb < NHB:
                stream.append(("spec", hb))
                hb += 1

            load_bs(0)
            for n in range(3):
                load_hb(n)
            # zero-fill PV bank once so evacuation of the unwritten
            # partition gaps (32h+17..32h+31) reads defined data
            guards["pv5"] = nc.vector.memset(T[0:128, 5, :], 0.0)

            fronts = {"spat": spat_front, "spec": spec_front}
            mids = {"spat": spat_mid, "spec": spec_mid}
            tails = {"spat": spat_tail, "spec": spec_tail}
            LAG = 3
            N = len(stream)
            for n in range(N + LAG):
                if n < N:
                    fronts[stream[n][0]](n, stream[n])
                if 1 <= n and n - 1 < N:
                    mids[stream[n - 1][0]](n - 1, stream[n - 1])
                if n >= LAG and n - LAG < N:
                    tails[stream[n - LAG][0]](n - LAG, stream[n - LAG])

    nc.compile()
    return nc


# ------------------------------------------------------------------ entry

def kernel(query, feat):
    from concourse.bass_utils import run_bass_kernel_spmd

    query = np.asarray(query, dtype=np.float32)
    feat = np.asarray(feat, dtype=np.float32)
    if "nc" not in _CACHE:
        _CACHE["nc"] = _build_program()
    nc = _CACHE["nc"]
    in_maps = _host_slices(query, feat)
    res = run_bass_kernel_spmd(nc, in_maps, core_ids=list(range(NCORES)))
    _CACHE["exec_time_ns"] = res.exec_time_ns
    _CACHE["res"] = res
    return _decode(res.results)


# ---------------------------------------------------- numpy device mirror

def _simulate_core(m):
    """Mirror of the device program in numpy, from prepped inputs to outputs."""
    kt = m["kt"].astype(np.float32)
    qt = m["qt"].astype(np.float32)
    vo = m["vo"].astype(np.float32)
    out_spat = np.zeros((NS, 128, HW), np.float32)
    for i in range(NS):
        for h in range(NH):
            pb, blk = (32 * h, 0) if h < 3 else (0, 1)
            sT = kt[i, pb:pb + 16, blk, :].T @ qt[i, pb:pb + 16, blk, :HW]
            pT = np.exp(SCALE * sT).astype(np.float32)   # [tpad, p]
            import ml_dtypes
            pT = pT.astype(ml_dtypes.bfloat16).astype(np.float32)
            acc = np.zeros((17, HW), np.float32)
            for t_c in range(5):
                acc += (vo[i, t_c, :, 17 * h:17 * h + 17].T
                        @ pT[128 * t_c:128 * (t_c + 1)])
            out_spat[i, 32 * h:32 * h + 17] = acc
    out_spec = np.zeros((NG, 128, 17), np.float32)  # 17 useful cols
    for g in range(NG):
        sT = m["km"][g].astype(np.float32).T @ m["qm"][g].astype(np.float32)
        pT = np.exp(SCALE * sT)
        import ml_dtypes
        pT = pT.astype(ml_dtypes.bfloat16).astype(np.float32)
        out_spec[g] = pT.T @ m["vom"][g].astype(np.float32)[:, :17]
    return dict(out_spat=out_spat, out_spec=out_spec)


def kernel_numpy(query, feat):
    """Host-only functional mirror (for layout validation)."""
    query = np.asarray(query, dtype=np.float32)
    feat = np.asarray(feat, dtype=np.float32)
    return _decode([_simulate_core(m) for m in _host_slices(query, feat)])


# revision 39
# speedup vs baseline: 1.1918x; 1.0672x over previous
"""Trainium2 Bass kernel for dual (spectral + spatial) multi-head cross-attention.

Reference computation (see problem):
  q, kv: [B=2, S=32, H=24, W=24, C=64], heads=4, head_dim=16, scale=0.25
  spectral: attention over S per (b, pixel, head)
  spatial:  attention over H*W per (b, band, head)
  out = x_spectral + x_spatial

Sharding (8 cores):
  spatial : (b, s) pairs, 64 total -> 8 per core
  spectral: (b, pixel) pairs, 1152 total -> 144 per core (x4 heads = 576 problems)

Device strategy (per core), all matmul operands bf16:
  spatial per (b,s): t and p padded 576->640 (5 chunks of 128 / p-chunks 512+128).
    Q/K live at SBUF partitions 32h..32h+15 (head h) so the 4 heads' score
    matmuls run as concurrent 32x128 PE row-tiles into separate PSUM banks
    (h -> bank h for the 512-chunk, bank 4 for the 128-chunk).  exp via ACT
    (scale folded) -> bf16 pt.  PV uses V+ones lhsT (17 cols) with out at
    PSUM partitions 32h (bank 5) so the 4 heads run as concurrent 128x32
    col-tiles; the ones column yields softmax denominators for free.
    Host divides + transposes.
  spectral: 4 problems ([32,16] attention) packed block-diagonally into one
    [21,128]x[21,128]->[128,128] matmul with mask rows baked into the operands
    (exp(-30) ~ 0 kills cross-problem terms); PV = [128,128]x[128,18] with
    block-diag V+ones.  Fixed PSUM banks 6,7.  Host extracts diagonal blocks.
"""
import sys

sys.path.insert(0, '/opt/trn_rl_repo')

import numpy as np

B, S, HH, WW, C = 2, 32, 24, 24, 64
NH, HD = 4, 16
SCALE = HD ** -0.5
HW = HH * WW                      # 576
HWP = 640                         # padded t / p extent (5 x 128)
NCORES = 8
NS = (B * S) // NCORES            # 8 spatial (b,s) problems per core
NPX = (B * HW) // NCORES          # 144 (b,pixel) pairs per core
NQ = NPX * NH                     # 576 spectral problems per core
NG = NQ // 4                      # 144 block-diag groups of 4 problems
NHB = NG // 8                     # 18 spectral half-batches (8 groups each)
MASK = 120.0                      # pre-scale mask magnitude (exp(-30) after scale)

PCW = [256, 256, 128]             # spatial p-chunks (sum 640, >=576 real)

DVE_PAR = 1                       # parity of chunks whose exp runs on DVE
                                  # (Schraudolph bf16-bit trick); -1 disables
SCH_A = 128.0 / float(np.log(2.0))
SCH_C = 5.5                       # minimax shift constant

_CACHE = {}


def _bf16(a):
    import ml_dtypes
    return np.asarray(a, dtype=np.float32).astype(ml_dtypes.bfloat16)


# ---------------------------------------------------------------- host prep

def _prep_spatial(q2, kv2):
    """q2, kv2: [NS, HW, C] fp32 for this core's (b,s) slice.
    Returns kt [NS,128,HWP], qt [NS,128,HWP], vo [NS,5,128,NH*17] (bf16).
    kt[i, 32h+d, t] = K_h[t, d];  qt[i, 32h+d, p] = Q_h[p, d];
    vo[i, tc, r, 17h+j] = V_h[128tc+r, j], ones at col 17h+16 (0 on pad rows).
    """
    ns = q2.shape[0]
    kt = np.zeros((ns, 128, HWP), np.float32)
    qt = np.zeros((ns, 128, HWP), np.float32)
    vo = np.zeros((ns, 5, 128, NH * 17), np.float32)
    kr = kv2.reshape(ns, HW, NH, HD)
    qr = q2.reshape(ns, HW, NH, HD)
    for h in range(NH):
        # PE row band h: head h at partitions 32h..32h+15
        kt[:, 32 * h:32 * h + 16, :HW] = kr[:, :, h, :].transpose(0, 2, 1)
        qt[:, 32 * h:32 * h + 16, :HW] = qr[:, :, h, :].transpose(0, 2, 1)
    for tc in range(5):
        t0, t1 = 128 * tc, min(128 * (tc + 1), HW)
        tw = t1 - t0
        blk = kr[:, t0:t1, :, :]                      # [ns,tw,h,d]
        for h in range(NH):
            vo[:, tc, :tw, 17 * h:17 * h + 16] = blk[:, :, h, :]
            vo[:, tc, :tw, 17 * h + 16] = 1.0
    return _bf16(kt), _bf16(qt), _bf16(vo)


def _prep_spectral(q1, kv1):
    """q1, kv1: [NPX, S, C] fp32 for this core's (b,px) slice.
    Problems q = px*4 + h; groups of 4 -> block-diag operands.
    Returns km [NG,21,128], qm [NG,21,128], vom [NG,128,18] (bf16)."""
    npx = q1.shape[0]
    kq = kv1.reshape(npx, S, NH, HD).transpose(0, 2, 1, 3).reshape(NQ, S, HD)
    qq = q1.reshape(npx, S, NH, HD).transpose(0, 2, 1, 3).reshape(NQ, S, HD)
    km = np.zeros((NG, 21, 128), np.float32)
    qm = np.zeros((NG, 21, 128), np.float32)
    vom = np.zeros((NG, 128, 18), np.float32)
    for j in range(4):
        # lhsT rows 0..15: d, cols j*32+t = K[t, d]
        km[:, :16, j * 32:(j + 1) * 32] = kq[j::4].transpose(0, 2, 1)
        qm[:, :16, j * 32:(j + 1) * 32] = qq[j::4].transpose(0, 2, 1)
        # mask rows: scores^T[m=t-stack, n=s-stack] += sum_i lhsT[16+i,m]*rhs[16+i,n]
        km[:, 16 + j, j * 32:(j + 1) * 32] = MASK      # lhsT indicator * M
        qm[:, 16 + j, j * 32:(j + 1) * 32] = 1.0       # rhs indicator
        km[:, 20, :] = -MASK                           # constant -M row
        qm[:, 20, :] = 1.0
        vom[:, j * 32:(j + 1) * 32, :16] = kq[j::4]
        vom[:, j * 32:(j + 1) * 32, 16] = 1.0
    return _bf16(km), _bf16(qm), _bf16(vom)


def _host_slices(query, feat):
    """Full inputs -> per-core input dicts (numpy)."""
    q4 = query.reshape(B * S, HW, C)        # (b,s) major
    f4 = feat.reshape(B * S, HW, C)
    q1 = query.transpose(0, 2, 3, 1, 4).reshape(B * HW, S, C)  # (b,px) major
    f1 = feat.transpose(0, 2, 3, 1, 4).reshape(B * HW, S, C)
    maps = []
    for c in range(NCORES):
        kt, qt, vo = _prep_spatial(q4[c * NS:(c + 1) * NS],
                                   f4[c * NS:(c + 1) * NS])
        km, qm, vom = _prep_spectral(q1[c * NPX:(c + 1) * NPX],
                                     f1[c * NPX:(c + 1) * NPX])
        maps.append(dict(kt=kt, qt=qt, vo=vo, km=km, qm=qm, vom=vom))
    return maps


def _decode(results):
    """Per-core outputs -> full [B,S,H,W,C]."""
    x_spat = np.zeros((B * S, HW, C), np.float32)
    x_spec = np.zeros((B * HW, S, C), np.float32)
    for c, r in enumerate(results):
        ospat = r["out_spat"]          # [NS, 128, HW]
        for h in range(NH):
            num = ospat[:, 32 * h:32 * h + 16, :]          # [NS, 16, 576]
            den = ospat[:, 32 * h + 16:32 * h + 17, :]     # [NS, 1, 576]
            x_spat[c * NS:(c + 1) * NS, :, 16 * h:16 * h + 16] = \
                (num / den).transpose(0, 2, 1)
        ospec = r["out_spec"]          # [NG, 128, 17]
        for j in range(4):
            # group g block j = (pixel g, head j)
            blk = ospec[:, j * 32:(j + 1) * 32, :]
            num = blk[:, :, :16]                            # [NG, 32, 16]
            den = blk[:, :, 16:17]
            x_spec[c * NPX:(c + 1) * NPX, :, 16 * j:16 * j + 16] = num / den
    full_spat = x_spat.reshape(B, S, HH, WW, C)
    full_spec = x_spec.reshape(B, HH, WW, S, C).transpose(0, 3, 1, 2, 4)
    return full_spat + full_spec


# ------------------------------------------------------------- device build

def _build_program():
    import concourse.bacc as bacc
    import concourse.tile as tile
    from concourse import mybir

    FP32 = mybir.dt.float32
    BF16 = mybir.dt.bfloat16
    I16 = mybir.dt.int16
    EXP = mybir.ActivationFunctionType.Exp
    A2 = float(SCALE * SCH_A)
    B2 = float(127 * 128 - SCH_C)

    from concourse.bass import _add_dep_helper

    nc = bacc.Bacc("TRN2", target_bir_lowering=False, debug=False)

    d_kt = nc.dram_tensor("kt", [NS, 128, HWP], BF16, kind="ExternalInput")
    d_qt = nc.dram_tensor("qt", [NS, 128, HWP], BF16, kind="ExternalInput")
    d_vo = nc.dram_tensor("vo", [NS, 5, 128, NH * 17], BF16, kind="ExternalInput")
    d_km = nc.dram_tensor("km", [NG, 21, 128], BF16, kind="ExternalInput")
    d_qm = nc.dram_tensor("qm", [NG, 21, 128], BF16, kind="ExternalInput")
    d_vom = nc.dram_tensor("vom", [NG, 128, 18], BF16, kind="ExternalInput")
    d_ospat = nc.dram_tensor("out_spat", [NS, 128, HW], FP32, kind="ExternalOutput")
    d_ospec = nc.dram_tensor("out_spec", [NG, 128, 17], FP32, kind="ExternalOutput")

    with tile.TileContext(nc) as tc:
        with (
            tc.tile_pool(name="cons", bufs=1) as cons,
            tc.tile_pool(name="io", bufs=3) as io,
            tc.tile_pool(name="spin", bufs=4) as spin,
            tc.tile_pool(name="pt", bufs=6) as ptp,
            tc.tile_pool(name="ps", bufs=1, space="PSUM") as ps,
        ):
            # persistent spatial V/ones operand
            vo_t = cons.tile([128, NS, 5, NH * 17], BF16)
            nc.sync.dma_start(vo_t[:], d_vo[:].rearrange("n tc p c -> p n tc c"))

            # all of PSUM as one tile.
            # banks 0-3: spatial scores (head h -> bank h, parity ping-pong
            #            in col halves 256*(n%2))
            # bank 4:    spare
            # bank 5:    spatial PV accumulators ([17]@partition 32h;
            #            pc0 -> cols 0-255, pc1 -> 256-511, pc2 -> 0-127)
            # banks 6,7: spectral scores / PV / out
            T = ps.tile([128, 8, 512], FP32, tag="T")

            kts, qts, sbos = {}, {}, {}
            _guard_init = {}
            kms, qms, voms = {}, {}, {}
            # explicit WAR guards: psum-region re-writers wait on the last
            # reader (ACT exp or DVE evacuation) of that region
            guards = {}

            def guarded(inst, key):
                dep = guards.pop(key, None)
                if dep is not None:
                    _add_dep_helper(inst.ins, dep.ins, sync=True,
                                    reason=f"WAR {key}")
                return inst

            def load_bs(i):
                kts[i] = io.tile([128, HWP], BF16, tag="kt", name=f"kt{i}")
                qts[i] = io.tile([128, HWP], BF16, tag="qt", name=f"qt{i}")
                nc.sync.dma_start(kts[i][:], d_kt[i])
                nc.sync.dma_start(qts[i][:], d_qt[i])
                sbos[i] = io.tile([128, HW], FP32, tag="so", name=f"so{i}")

            def load_hb(n):
                if n >= NHB:
                    return
                kms[n] = spin.tile([128, 8, 128], BF16, tag="km", name=f"km{n}")
                qms[n] = spin.tile([128, 8, 128], BF16, tag="qm", name=f"qm{n}")
                voms[n] = spin.tile([128, 8, 18], BF16, tag="vom", name=f"vom{n}")
                sl = np.s_[8 * n:8 * (n + 1)]
                nc.sync.dma_start(kms[n][0:21], d_km[sl].rearrange("g r c -> r g c"))
                nc.sync.dma_start(qms[n][0:21], d_qm[sl].rearrange("g r c -> r g c"))
                nc.sync.dma_start(voms[n][:], d_vom[sl].rearrange("g r c -> r g c"))

            # ---- spatial: chunk = (i, pc, tc); parity = n%2 ----
            # PV accumulator cols in bank 5 per pc:
            PVC = [0, 256, 0]                 # col base
            PVW = [256, 256, 128]             # accum width (pc2 incl junk)
            SBC = [0, 256, 512]               # sbos col base
            SBW = [256, 256, 64]              # real cols evacuated

            def spat_front(n, item):
                _, i, pc, t_c = item
                if pc == 1 and t_c == 0 and i + 1 < NS:
                    load_bs(i + 1)
                p0 = sum(PCW[:pc])
                pw = PCW[pc]
                par = n % 2
                for h in range(NH):
                    mm = nc.tensor.matmul(
                        T[0:128, h, 256 * par:256 * par + pw],
                        kts[i][32 * h:32 * h + 16, 128 * t_c:128 * (t_c + 1)],
                        qts[i][32 * h:32 * h + 16, p0:p0 + pw],
                        start=True, stop=True, skip_group_check=True,
                        tile_position=(32 * h, 0))
                    if h == 0:
                        guarded(mm, f"s{par}")

            pts_of = {}

            def spat_mid(n, item):
                _, i, pc, t_c = item
                pw = PCW[pc]
                par = n % 2
                src = T[0:128, 0:4, 256 * par:256 * par + pw]
                if par == DVE_PAR:
                    # Schraudolph exp on DVE: bf16 bits = round(x*A2 + B2)
                    pt_t = ptp.tile([128, 4, 256], I16, tag="pti", name=f"pt{n}")
                    ex = nc.vector.tensor_scalar(
                        pt_t[:, :, 0:pw], src, A2, B2,
                        mybir.AluOpType.mult, mybir.AluOpType.add)
                else:
                    pt_t = ptp.tile([128, 4, 256], BF16, tag="pt", name=f"pt{n}")
                    ex = nc.scalar.activation(
                        pt_t[:, :, 0:pw], src, EXP, scale=float(SCALE))
                pts_of[n] = pt_t
                guards[f"s{par}"] = ex

            def spat_tail(n, item):
                _, i, pc, t_c = item
                pw = PCW[pc]
                par = n % 2
                pt_t = pts_of.pop(n)
                first = None
                for h in range(NH):
                    rhs = pt_t[0:128, h, 0:pw]
                    if par == DVE_PAR:
                        rhs = rhs.bitcast(BF16)
                    mm = nc.tensor.matmul(
                        T[32 * h:32 * h + 17, 5, PVC[pc]:PVC[pc] + PVW[pc]],
                        vo_t[0:128, i, t_c, 17 * h:17 * h + 17],
                        rhs,
                        start=(t_c == 0), stop=(t_c == 4),
                        skip_group_check=True,
                        tile_position=(0, 32 * h))
                    if t_c == 0:
                        # start=True clears the whole bank's has_written bits;
                        # only h0 may clear, h1-3 ride on cleared bits but
                        # must issue after h0 (bank clear order).
                        if h == 0:
                            first = mm
                            guarded(mm, "pv5")
                        else:
                            _add_dep_helper(mm.ins, first.ins, sync=False,
                                            reason="bank clear order")
                if t_c == 4:
                    ev = nc.vector.tensor_copy(
                        sbos[i][0:128, SBC[pc]:SBC[pc] + SBW[pc]],
                        T[0:128, 5, PVC[pc]:PVC[pc] + SBW[pc]])
                    guards["pv5"] = ev
                    if pc == 2:
                        nc.sync.dma_start(d_ospat[i], sbos[i][:])

            # ---- spectral: half-batch hb of 8 groups in banks 6,7 ----

            def spec_front(n, item):
                hb = item[1]
                load_hb(hb + 3)
                for g in range(8):
                    mm = nc.tensor.matmul(
                        T[:, 6 + g // 4, (g % 4) * 128:(g % 4) * 128 + 128],
                        kms[hb][0:21, g, :], qms[hb][0:21, g, :],
                        start=True, stop=True, skip_group_check=True)
                    if g == 0:
                        guarded(mm, "spec6")
                    elif g == 4:
                        guarded(mm, "spec7")

            def spec_mid(n, item):
                pts = ptp.tile([128, 2, 512], BF16, tag="pts", name=f"pts{n}")
                pts_of[n] = pts
                act = nc.scalar.activation(pts[:], T[:, 6:8, :],
                                           EXP, scale=float(SCALE))
                guards["spec7"] = act

            def spec_tail(n, item):
                hb = item[1]
                pts = pts_of.pop(n)
                for g in range(8):
                    # 18-col pitch keeps psum dst 8B-aligned
                    nc.tensor.matmul(
                        T[:, 6, 18 * g:18 * g + 18],
                        pts[:, g // 4, (g % 4) * 128:(g % 4) * 128 + 128],
                        voms[hb][:, g, :],
                        start=True, stop=True, skip_group_check=True)
                so_t = spin.tile([128, 8, 17], FP32, tag="spo", name=f"spo{n}")
                guards["spec6"] = nc.vector.tensor_copy(
                    so_t[:], T[:, 6, 0:144].rearrange(
                        "p (g c) -> p g c", c=18)[:, :, 0:17])
                nc.sync.dma_start(
                    d_ospec[8 * hb:8 * (hb + 1)].rearrange("g r c -> r g c"),
                    so_t[:])

            # ---- build interleaved chunk stream ----
            spat = [("spat", i, pc, t_c) for i in range(NS)
                    for pc in range(3) for t_c in range(5)]
            stream = []
            hb = 0
            for idx, itm in enumerate(spat):
                stream.append(itm)
                if hb < NHB and (idx + 1) * NHB >= (hb + 1) * len(spat):
                    stream.append(("spec", hb))
                    hb += 1
            while hb < NHB:
                stream.append(("spec", hb))
                hb += 1

            load_bs(0)
            for n in range(3):
                load_hb(n)
            # zero-fill PV bank once so evacuation of the unwritten
            # partition gaps (32h+17..32h+31) reads defined data
            guards["pv5"] = nc.vector.memset(T[0:128, 5, :], 0.0)

            fronts = {"spat": spat_front, "spec": spec_front}
            mids = {"spat": spat_mid, "spec": spec_mid}
            tails = {"spat": spat_tail, "spec": spec_tail}
            LAG = 3
            N = len(stream)
            for n in range(N + LAG):
                if n < N:
                    fronts[stream[n][0]](n, stream[n])
                if 1 <= n and n - 1 < N:
                    mids[stream[n - 1][0]](n - 1, stream[n - 1])
                if n >= LAG and n - LAG < N:
                    tails[stream[n - LAG][0]](n - LAG, stream[n - LAG])

    nc.compile()
    return nc


# ------------------------------------------------------------------ entry

def kernel(query, feat):
    from concourse.bass_utils import run_bass_kernel_spmd

    query = np.asarray(query, dtype=np.float32)
    feat = np.asarray(feat, dtype=np.float32)
    if "nc" not in _CACHE:
        _CACHE["nc"] = _build_program()
    nc = _CACHE["nc"]
    in_maps = _host_slices(query, feat)
    res = run_bass_kernel_spmd(nc, in_maps, core_ids=list(range(NCORES)))
    _CACHE["exec_time_ns"] = res.exec_time_ns
    _CACHE["res"] = res
    return _decode(res.results)


# ---------------------------------------------------- numpy device mirror

def _simulate_core(m):
    """Mirror of the device program in numpy, from prepped inputs to outputs."""
    kt = m["kt"].astype(np.float32)
    qt = m["qt"].astype(np.float32)
    vo = m["vo"].astype(np.float32)
    out_spat = np.zeros((NS, 128, HW), np.float32)
    for i in range(NS):
        for h in range(NH):
            sT = kt[i, 32 * h:32 * h + 16, :].T @ qt[i, 32 * h:32 * h + 16, :HW]
            pT = np.exp(SCALE * sT).astype(np.float32)   # [tpad, p]
            import ml_dtypes
            pT = pT.astype(ml_dtypes.bfloat16).astype(np.float32)
            acc = np.zeros((17, HW), np.float32)
            for t_c in range(5):
                acc += (vo[i, t_c, :, 17 * h:17 * h + 17].T
                        @ pT[128 * t_c:128 * (t_c + 1)])
            out_spat[i, 32 * h:32 * h + 17] = acc
    out_spec = np.zeros((NG, 128, 17), np.float32)  # 17 useful cols
    for g in range(NG):
        sT = m["km"][g].astype(np.float32).T @ m["qm"][g].astype(np.float32)
        pT = np.exp(SCALE * sT)
        import ml_dtypes
        pT = pT.astype(ml_dtypes.bfloat16).astype(np.float32)
        out_spec[g] = pT.T @ m["vom"][g].astype(np.float32)[:, :17]
    return dict(out_spat=out_spat, out_spec=out_spec)


def kernel_numpy(query, feat):
    """Host-only functional mirror (for layout validation)."""
    query = np.asarray(query, dtype=np.float32)
    feat = np.asarray(feat, dtype=np.float32)
    return _decode([_simulate_core(m) for m in _host_slices(query, feat)])
